# revision 2
# baseline (speedup 1.0000x reference)
"""Fused multi-head attention on 8 Trainium2 NeuronCores.

Problem: x[2,2048,1024] -> qkv proj (16 heads, hd=64) -> softmax attention
-> out proj.  Sharding: tensor parallel over heads, 2 heads per core.
Each core computes q/k/v for its 2 heads, full attention for its
4 (batch, head) pairs, and the partial out-projection contribution of its
128 head-dims.  Host sums the 8 partial outputs and adds out_b.

Layouts on device (per core):
  xT    [1024, 4096]  bf16   hidden on partitions, tokens free (b-major)
  qkvT  [128, 4096]   bf16   per group; head A dims on partitions 0-63, B on 64-127
  scores^T in PSUM: [k-tile 128, q 512] per head, heads packed side by side
  p = exp(scores/8) in SBUF bf16 (no max subtraction: |scores/8| < ~3)
  PV: lhsT = v_aug [k-tile 128, 65] (col 64 = ones -> colsum row), rhs = p
  o_aug^T [65, q] accumulated in PSUM over 16 k-tiles
  out-proj per head: lhsT = o_aug^T [65, t-tile 128], rhs = woT_aug [65, 1025]
    (row 64 of rhs is zero in cols 0-1023 and 1 in col 1024, so col 1024 of
     the output is the per-token colsum, transposed to the partition dim)
  y = recipA * y_A + recipB * y_B on DVE (per-partition scalars), f32 out.
"""

import sys
import types
import numpy as np
import ml_dtypes

import concourse.bass as bass
import concourse.tile as tile
from concourse import bacc, mybir

BF16 = mybir.dt.bfloat16
F32 = mybir.dt.float32
BF16_NP = ml_dtypes.bfloat16

B, S, H, NH, HD = 2, 2048, 1024, 16, 64
T = B * S               # 4096 tokens, b-major
NCORES = 8
HPC = NH // NCORES      # heads per core = 2
DPC = HPC * HD          # head dims per core = 128
KT = 128                # keys per k-tile
NKT = S // KT           # 16
QC = 512                # query chunk
NQC = S // QC           # 4
TCH = 512               # token chunk for qkv proj
NTCH = T // TCH         # 8
HKT = H // 128          # hidden k-tiles = 8
VS = 80                 # v_aug stride per k-tile (64 v + 1 ones + pad; 160B aligned)
EXPSCALE = 1.0 / np.sqrt(HD)

_CACHED = {}


def _build_nc():
    nc = bacc.Bacc(None, target_bir_lowering=False, debug=False)
    xT = nc.dram_tensor("xT", [H, T], BF16, kind="ExternalInput").ap()
    wqkvT = nc.dram_tensor("wqkvT", [H, 3 * DPC], BF16, kind="ExternalInput").ap()
    bqkv = nc.dram_tensor("bqkv", [DPC, 3], F32, kind="ExternalInput").ap()
    woTa = nc.dram_tensor("woTa", [HPC, HD + 1, H + 1], BF16, kind="ExternalInput").ap()
    out = nc.dram_tensor("out", [T, H], F32, kind="ExternalOutput").ap()

    EXP = mybir.ActivationFunctionType.Exp
    MULT = mybir.AluOpType.mult
    ADD = mybir.AluOpType.add

    with tile.TileContext(nc) as tc:
        with (
            tc.tile_pool(name="const", bufs=1) as constp,
            tc.tile_pool(name="xw", bufs=1) as xwp,
            tc.tile_pool(name="qkv", bufs=1) as qkvp,
            tc.tile_pool(name="vaug", bufs=1) as vaugp,
            tc.tile_pool(name="oT", bufs=4) as oTp,
            tc.tile_pool(name="p", bufs=3) as pp,
            tc.tile_pool(name="ysb", bufs=3) as ysbp,
            tc.tile_pool(name="small", bufs=8) as smallp,
            tc.tile_pool(name="ps_s", bufs=2, space="PSUM") as ps_s,
            tc.tile_pool(name="ps_o", bufs=2, space="PSUM") as ps_o,
            tc.tile_pool(name="ps_y", bufs=2, space="PSUM") as ps_y,
        ):
            # ---- constants / weights in ----
            bias_sb = constp.tile([DPC, 3], F32, tag="bias")
            nc.sync.dma_start(bias_sb[:], bqkv[:])
            wo_sb = [constp.tile([HD + 1, H + 1], BF16, name=f"wo{h}", tag=f"wo{h}") for h in range(HPC)]
            for h in range(HPC):
                nc.sync.dma_start(wo_sb[h][:], woTa[h])

            # ---- x and qkv weights in ----
            xT_sb = [xwp.tile([128, T], BF16, name=f"xsb{k}", tag=f"x{k}") for k in range(HKT)]
            wq_sb = [xwp.tile([128, 3 * DPC], BF16, name=f"wsb{k}", tag=f"w{k}") for k in range(HKT)]
            for k in range(HKT):
                nc.sync.dma_start(wq_sb[k][:], wqkvT[k * 128:(k + 1) * 128, :])
                nc.sync.dma_start(xT_sb[k][:], xT[k * 128:(k + 1) * 128, :])

            # ---- qkv projection ----
            # order v, q, k so the v transposes can start early
            qkvT_sb = {
                fg: qkvp.tile([128, T], BF16, name=f"qkvsb{fg}", tag=f"qkv{fg}") for fg in range(3)
            }
            for fg in (2, 0, 1):  # v, q, k
                for t in range(NTCH):
                    ps = ps_s.tile([128, TCH], F32, tag="s")
                    for k in range(HKT):
                        nc.tensor.matmul(
                            ps[:],
                            lhsT=wq_sb[k][:, fg * DPC:(fg + 1) * DPC],
                            rhs=xT_sb[k][:, t * TCH:(t + 1) * TCH],
                            start=(k == 0),
                            stop=(k == HKT - 1),
                        )
                    nc.vector.tensor_scalar_add(
                        qkvT_sb[fg][:, t * TCH:(t + 1) * TCH], ps[:],
                        bias_sb[:, fg:fg + 1],
                    )

            qT_sb, kT_sb, vT_sb = qkvT_sb[0], qkvT_sb[1], qkvT_sb[2]

            # ---- v_aug: DMA-transpose v^T into [k-tile 128, 64] tiles + ones col
            vaug = {}
            for b in range(B):
                for h in range(HPC):
                    va = vaugp.tile([128, NKT * VS], BF16, name=f"va{b}{h}", tag=f"va{b}{h}")
                    nc.vector.memset(va[:], 1.0)
                    for kt in range(NKT):
                        nc.sync.dma_start(
                            va[:, kt * VS:kt * VS + HD],
                            vT_sb[h * HD:(h + 1) * HD,
                                  b * S + kt * KT:b * S + (kt + 1) * KT],
                            transpose=True,
                        )
                    vaug[(b, h)] = va

            # ---- attention + out-projection, per (batch, q-chunk) ----
            for b in range(B):
                for qc in range(NQC):
                    q0 = b * S + qc * QC
                    o_ps = [ps_o.tile([HD + 1, QC], F32, name=f"ops{_h}", tag="o") for _h in range(HPC)]
                    for kt in range(NKT):
                        s_ps = ps_s.tile([128, HPC * QC], F32, tag="s")
                        for h in range(HPC):
                            # scores^T [k-tile, q]: lhsT = k^T [64, 128], rhs = q^T [64, 512]
                            nc.tensor.matmul(
                                s_ps[:, h * QC:(h + 1) * QC],
                                lhsT=kT_sb[h * HD:(h + 1) * HD,
                                           b * S + kt * KT:b * S + (kt + 1) * KT],
                                rhs=qT_sb[h * HD:(h + 1) * HD, q0:q0 + QC],
                                start=True, stop=True,
                            )
                        p_sb = pp.tile([128, HPC * QC], BF16, tag="p")
                        nc.scalar.activation(p_sb[:], s_ps[:], EXP, scale=float(EXPSCALE))
                        for h in range(HPC):
                            nc.tensor.matmul(
                                o_ps[h][:],
                                lhsT=vaug[(b, h)][:, kt * VS:kt * VS + HD + 1],
                                rhs=p_sb[:, h * QC:(h + 1) * QC],
                                start=(kt == 0), stop=(kt == NKT - 1),
                            )
                    # o_aug^T -> SBUF bf16 (rows 0-63 = o^T, row 64 = colsum)
                    oT = [oTp.tile([HD + 1, QC], BF16, name=f"oT{_h}", tag="oT") for _h in range(HPC)]
                    for h in range(HPC):
                        nc.vector.tensor_copy(oT[h][:], o_ps[h][:])

                    # out-projection for this token chunk (4 t-tiles of 128)
                    for tt in range(4):
                        tsl = slice(tt * KT, (tt + 1) * KT)
                        # per-head colsum^T via the unit column of woTa
                        ct_ps = [ps_o.tile([128, 4], F32, name=f"ct{_h}", tag="o") for _h in range(HPC)]
                        rT = smallp.tile([128, HPC], F32, tag="rT")
                        for h in range(HPC):
                            nc.tensor.matmul(
                                ct_ps[h][:, 0:1],
                                lhsT=oT[h][:, tsl],
                                rhs=wo_sb[h][:, H:H + 1],
                                start=True, stop=True,
                            )
                            nc.vector.reciprocal(rT[:, h:h + 1], ct_ps[h][:, 0:1])
                        for ec in range(2):
                            esl = slice(ec * 512, (ec + 1) * 512)
                            y_ps = [ps_y.tile([128, 512], F32, name=f"yps{_h}", tag="y") for _h in range(HPC)]
                            for h in range(HPC):
                                nc.tensor.matmul(
                                    y_ps[h][:],
                                    lhsT=oT[h][:, tsl],
                                    rhs=wo_sb[h][:, esl],
                                    start=True, stop=True,
                                )
                            y_sb = ysbp.tile([128, 512], F32, tag="ysb")
                            nc.vector.tensor_scalar_mul(y_sb[:], y_ps[0][:], rT[:, 0:1])
                            nc.vector.scalar_tensor_tensor(
                                y_sb[:], y_ps[1][:], rT[:, 1:2], y_sb[:],
                                op0=MULT, op1=ADD,
                            )
                            nc.sync.dma_start(
                                out[q0 + tt * KT:q0 + (tt + 1) * KT, esl], y_sb[:]
                            )
    nc.compile()
    return nc


def _get_nc():
    if "nc" not in _CACHED:
        _CACHED["nc"] = _build_nc()
    return _CACHED["nc"]


def _host_prep(x, qkv_w, qkv_b, out_w):
    x = np.asarray(x, dtype=np.float32)
    qkv_w = np.asarray(qkv_w, dtype=np.float32)
    qkv_b = np.asarray(qkv_b, dtype=np.float32)
    out_w = np.asarray(out_w, dtype=np.float32)

    xT = np.ascontiguousarray(x.reshape(T, H).T).astype(BF16_NP)
    in_maps = []
    for c in range(NCORES):
        r = slice(128 * c, 128 * c + 128)
        wq = qkv_w[0 * H:][r.start:r.stop] if False else qkv_w[128 * c:128 * c + 128]
        wk = qkv_w[H + 128 * c:H + 128 * c + 128]
        wv = qkv_w[2 * H + 128 * c:2 * H + 128 * c + 128]
        wqkvT = np.ascontiguousarray(np.concatenate([wq, wk, wv], 0).T).astype(BF16_NP)
        bq = np.stack(
            [qkv_b[fg * H + 128 * c:fg * H + 128 * c + 128] for fg in range(3)],
            axis=1,
        ).astype(np.float32)
        woTa = np.zeros((HPC, HD + 1, H + 1), np.float32)
        for h in range(HPC):
            g = HPC * c + h
            woTa[h, 0:HD, 0:H] = out_w[:, g * HD:(g + 1) * HD].T
            woTa[h, HD, H] = 1.0
        in_maps.append({
            "xT": xT,
            "wqkvT": wqkvT,
            "bqkv": np.ascontiguousarray(bq),
            "woTa": woTa.astype(BF16_NP),
        })
    return in_maps


def _run(in_maps, trace=False):
    # The image's antenv lacks axon_hooks; register the NTFF profile hook so
    # run_bass_kernel_spmd(trace=True) can report exec_time_ns.
    if trace and "antenv.axon_hooks" not in sys.modules:
        try:
            import trn_agent_boot.trn_boot as _tb
            _hook = _tb._ntff_profile_via_ctypes("/opt/axon/libaxon_pjrt.so")
            _m = types.ModuleType("antenv.axon_hooks")
            _m.get_axon_ntff_profile_hook = lambda: _hook
            sys.modules["antenv.axon_hooks"] = _m
        except Exception:
            trace = False
    from concourse.bass_utils import run_bass_kernel_spmd

    nc = _get_nc()
    res = run_bass_kernel_spmd(nc, in_maps, core_ids=list(range(NCORES)), trace=trace)
    return res


def kernel(x, qkv_w, qkv_b, out_w, out_b):
    in_maps = _host_prep(x, qkv_w, qkv_b, out_w)
    res = _run(in_maps, trace=False)
    total = np.zeros((T, H), np.float32)
    for c in range(NCORES):
        total += res.results[c]["out"]
    total += np.asarray(out_b, dtype=np.float32)[None, :]
    return total.reshape(B, S, H)


# revision 5
# speedup vs baseline: 1.1905x; 1.1905x over previous
"""Fused multi-head attention on 8 Trainium2 NeuronCores.

Problem: x[2,2048,1024] -> qkv proj (16 heads, hd=64) -> softmax attention
-> out proj.  Sharding: tensor parallel over heads, 2 heads per core.
Each core computes q/k/v for its 2 heads, full attention for its
4 (batch, head) pairs, and the partial out-projection contribution of its
128 head-dims.  Host sums the 8 partial outputs and adds out_b.

Layouts on device (per core):
  xT    [1024, 4096]  bf16   hidden on partitions, tokens free (b-major)
  qkvT  [128, 4096]   bf16   per group; head A dims on partitions 0-63, B on 64-127
  scores^T in PSUM: [k-tile 128, q 512] per head, heads packed side by side
  p = exp(scores/8) in SBUF bf16 (no max subtraction: |scores/8| < ~3)
  PV: lhsT = v_aug [k-tile 128, 65] slices of a combined per-batch tile
      holding [pad | onesA | v(A,B) transposed | onesB]; the ones column
      produces the softmax colsum as an extra row of o (row 0 for head A,
      row 64 for head B).
  out-proj per head: lhsT = o_aug^T [65, t-tile 128], rhs = woT_aug [65, 1025]
    (the unit row of rhs is zero in cols 0-1023 and 1 in col 1024, so col
     1024 of the output is the per-token colsum, transposed to partitions)
  y = recipA * y_A + recipB * y_B on DVE (per-partition scalars), f32 out.

The attention loop is software-pipelined: PV lags one k-tile behind
scores/exp, and the previous q-chunk's out-projection epilogue is
interleaved into the current chunk's k-loop so the PE never waits on the
scalar engine's exp.
"""

import sys
import types
import numpy as np
import ml_dtypes

import concourse.bass as bass
import concourse.tile as tile
from concourse import bacc, mybir

BF16 = mybir.dt.bfloat16
F32 = mybir.dt.float32
BF16_NP = ml_dtypes.bfloat16

B, S, H, NH, HD = 2, 2048, 1024, 16, 64
T = B * S               # 4096 tokens, b-major
NCORES = 8
HPC = NH // NCORES      # heads per core = 2
DPC = HPC * HD          # head dims per core = 128
KT = 128                # keys per k-tile
NKT = S // KT           # 16
QC = 512                # query chunk
NQC = S // QC           # 4
HKT = H // 128          # hidden k-tiles = 8
VS = 160                # v_aug stride per k-tile (pad15|onesA|v 128|onesB|pad15)
VOFF = 16               # col offset of the transposed v block within a stride
EXPSCALE = 1.0 / np.sqrt(HD)

_CACHED = {}


def _build_nc():
    nc = bacc.Bacc(None, target_bir_lowering=False, debug=False)
    xT = nc.dram_tensor("xT", [H, T], BF16, kind="ExternalInput").ap()
    wqkvT = nc.dram_tensor("wqkvT", [H, 3 * DPC], BF16, kind="ExternalInput").ap()
    bqkv = nc.dram_tensor("bqkv", [DPC, 3], F32, kind="ExternalInput").ap()
    woTa = nc.dram_tensor("woTa", [HPC, HD + 1, H + 1], BF16, kind="ExternalInput").ap()
    out = nc.dram_tensor("out", [T, H], F32, kind="ExternalOutput").ap()

    EXP = mybir.ActivationFunctionType.Exp
    MULT = mybir.AluOpType.mult
    ADD = mybir.AluOpType.add

    with tile.TileContext(nc) as tc:
        with (
            tc.tile_pool(name="const", bufs=1) as constp,
            tc.tile_pool(name="xw", bufs=1) as xwp,
            tc.tile_pool(name="qkv", bufs=1) as qkvp,
            tc.tile_pool(name="vaug", bufs=1) as vaugp,
            tc.tile_pool(name="oT", bufs=4) as oTp,
            tc.tile_pool(name="p", bufs=3) as pp,
            tc.tile_pool(name="ysb", bufs=3) as ysbp,
            tc.tile_pool(name="small", bufs=4) as smallp,
            tc.tile_pool(name="ps", bufs=2, space="PSUM") as psp,
        ):
            # ---- constants / weights in ----
            bias_sb = constp.tile([DPC, 3], F32, tag="bias")
            nc.sync.dma_start(bias_sb[:], bqkv[:])
            wo_sb = [
                constp.tile([HD + 1, H + 1], BF16, name=f"wo{h}", tag=f"wo{h}")
                for h in range(HPC)
            ]
            for h in range(HPC):
                nc.sync.dma_start(wo_sb[h][:], woTa[h])

            # ---- x and qkv weights in ----
            xT_sb = [xwp.tile([128, T], BF16, name=f"xsb{k}", tag=f"x{k}")
                     for k in range(HKT)]
            wq_sb = [xwp.tile([128, 3 * DPC], BF16, name=f"wsb{k}", tag=f"w{k}")
                     for k in range(HKT)]
            for k in range(HKT):
                nc.sync.dma_start(wq_sb[k][:], wqkvT[k * 128:(k + 1) * 128, :])
                nc.sync.dma_start(xT_sb[k][:], xT[k * 128:(k + 1) * 128, :])

            # vaug tiles (memset to 1.0 early so the ones columns are ready)
            vaug = {}
            for b in range(B):
                va = vaugp.tile([128, NKT * VS], BF16, name=f"va{b}", tag=f"va{b}")
                nc.vector.memset(va[:], 1.0)
                vaug[b] = va

            # ---- qkv projection (weight-stationary: kt outer, tc inner) ----
            qkvT_sb = {
                fg: qkvp.tile([128, T], BF16, name=f"qkvsb{fg}", tag=f"qkv{fg}")
                for fg in range(3)
            }

            def qkv_group(fg):
                tiles = [
                    psp.tile([128, 1024], F32, name=f"qp{fg}a", tag="s"),
                    psp.tile([128, 1024], F32, name=f"qp{fg}b", tag="s"),
                    psp.tile([128, 512], F32, name=f"qp{fg}c", tag="o"),
                    psp.tile([128, 512], F32, name=f"qp{fg}d", tag="o"),
                    psp.tile([128, 512], F32, name=f"qp{fg}e", tag="y"),
                    psp.tile([128, 512], F32, name=f"qp{fg}f", tag="y"),
                ]

                def tc_slice(t):
                    if t < 4:
                        return tiles[t // 2][:, (t % 2) * 512:(t % 2) * 512 + 512]
                    return tiles[2 + t - 4][:]

                for k in range(HKT):
                    for t in range(8):
                        nc.tensor.matmul(
                            tc_slice(t),
                            lhsT=wq_sb[k][:, fg * DPC:(fg + 1) * DPC],
                            rhs=xT_sb[k][:, t * 512:(t + 1) * 512],
                            start=(k == 0),
                            stop=(k == HKT - 1),
                        )
                for t in range(8):
                    nc.vector.tensor_scalar_add(
                        qkvT_sb[fg][:, t * 512:(t + 1) * 512], tc_slice(t),
                        bias_sb[:, fg:fg + 1],
                    )

            qkv_group(2)  # v first so transposes can start early
            # v_aug transposes: per head, [64,128] blocks on the sync ring
            for b in range(B):
                for kt in range(NKT):
                    for h in range(HPC):
                        nc.sync.dma_start(
                            vaug[b][:, kt * VS + VOFF + h * HD:
                                      kt * VS + VOFF + (h + 1) * HD],
                            qkvT_sb[2][h * HD:(h + 1) * HD,
                                       b * S + kt * KT:b * S + (kt + 1) * KT],
                            transpose=True,
                        )
            qkv_group(0)  # q
            qkv_group(1)  # k

            qT_sb, kT_sb = qkvT_sb[0], qkvT_sb[1]

            def va_lhsT(b, h, kt):
                # head 0: [onesA | vA] cols 15..79 ; head 1: [vB | onesB] 80..144
                c0 = kt * VS + (15 if h == 0 else 80)
                return vaug[b][:, c0:c0 + HD + 1]

            # ---- attention + pipelined out-projection epilogue ----
            pending = []  # epilogue steps of the previous (b, qc)

            def make_epilogue(b, qc, oT):
                q0 = b * S + qc * QC
                state = {}

                def ct_step():
                    ct = [psp.tile([128, 4], F32, name=f"ct{b}{qc}{h}", tag="y")
                          for h in range(HPC)]
                    rT = smallp.tile([128, 2 * 4], F32, name=f"rT{b}{qc}", tag="rT")
                    for h in range(HPC):
                        for tt in range(4):
                            nc.tensor.matmul(
                                ct[h][:, tt:tt + 1],
                                lhsT=oT[h][:, tt * KT:(tt + 1) * KT],
                                rhs=wo_sb[h][:, H:H + 1],
                                start=True, stop=True,
                            )
                        nc.vector.reciprocal(rT[:, h * 4:h * 4 + 4], ct[h][:])
                    state["rT"] = rT

                steps = [ct_step]

                def y_step(tt, ec):
                    def run():
                        rT = state["rT"]
                        y_ps = [
                            psp.tile([128, 512], F32, name=f"y{b}{qc}{tt}{ec}{h}",
                                     tag="y")
                            for h in range(HPC)
                        ]
                        for h in range(HPC):
                            nc.tensor.matmul(
                                y_ps[h][:],
                                lhsT=oT[h][:, tt * KT:(tt + 1) * KT],
                                rhs=wo_sb[h][:, ec * 512:(ec + 1) * 512],
                                start=True, stop=True,
                            )
                        y_sb = ysbp.tile([128, 512], F32, name=f"ys{b}{qc}{tt}{ec}",
                                         tag="ysb")
                        nc.vector.tensor_scalar_mul(
                            y_sb[:], y_ps[0][:], rT[:, 0 * 4 + tt:0 * 4 + tt + 1])
                        nc.vector.scalar_tensor_tensor(
                            y_sb[:], y_ps[1][:], rT[:, 1 * 4 + tt:1 * 4 + tt + 1],
                            y_sb[:], op0=MULT, op1=ADD,
                        )
                        nc.sync.dma_start(
                            out[q0 + tt * KT:q0 + (tt + 1) * KT,
                                ec * 512:(ec + 1) * 512],
                            y_sb[:],
                        )
                    return run

                for tt in range(4):
                    for ec in range(2):
                        steps.append(y_step(tt, ec))
                return steps

            for b in range(B):
                for qc in range(NQC):
                    q0 = b * S + qc * QC
                    o_ps = [psp.tile([HD + 1, QC], F32, name=f"o{b}{qc}{h}", tag="o")
                            for h in range(HPC)]
                    p_tiles = []
                    for kt in range(NKT):
                        s_ps = psp.tile([128, HPC * QC], F32, tag="s")
                        for h in range(HPC):
                            nc.tensor.matmul(
                                s_ps[:, h * QC:(h + 1) * QC],
                                lhsT=kT_sb[h * HD:(h + 1) * HD,
                                           b * S + kt * KT:b * S + (kt + 1) * KT],
                                rhs=qT_sb[h * HD:(h + 1) * HD, q0:q0 + QC],
                                start=True, stop=True,
                            )
                        p_sb = pp.tile([128, HPC * QC], BF16, tag="p")
                        nc.scalar.activation(p_sb[:], s_ps[:], EXP,
                                             scale=float(EXPSCALE))
                        p_tiles.append(p_sb)
                        if pending:
                            pending.pop(0)()
                        if kt >= 1:
                            for h in range(HPC):
                                nc.tensor.matmul(
                                    o_ps[h][:],
                                    lhsT=va_lhsT(b, h, kt - 1),
                                    rhs=p_tiles[kt - 1][:, h * QC:(h + 1) * QC],
                                    start=(kt - 1 == 0), stop=False,
                                )
                    for h in range(HPC):
                        nc.tensor.matmul(
                            o_ps[h][:],
                            lhsT=va_lhsT(b, h, NKT - 1),
                            rhs=p_tiles[NKT - 1][:, h * QC:(h + 1) * QC],
                            start=False, stop=True,
                        )
                    oT = [oTp.tile([HD + 1, QC], BF16, name=f"oT{b}{qc}{h}", tag="oT")
                          for h in range(HPC)]
                    for h in range(HPC):
                        nc.vector.tensor_copy(oT[h][:], o_ps[h][:])
                    while pending:
                        pending.pop(0)()
                    pending = make_epilogue(b, qc, oT)
            while pending:
                pending.pop(0)()
    nc.compile()
    return nc


def _get_nc():
    if "nc" not in _CACHED:
        _CACHED["nc"] = _build_nc()
    return _CACHED["nc"]


def _host_prep(x, qkv_w, qkv_b, out_w):
    x = np.asarray(x, dtype=np.float32)
    qkv_w = np.asarray(qkv_w, dtype=np.float32)
    qkv_b = np.asarray(qkv_b, dtype=np.float32)
    out_w = np.asarray(out_w, dtype=np.float32)

    xT = np.ascontiguousarray(x.reshape(T, H).T).astype(BF16_NP)
    in_maps = []
    for c in range(NCORES):
        wq = qkv_w[128 * c:128 * c + 128]
        wk = qkv_w[H + 128 * c:H + 128 * c + 128]
        wv = qkv_w[2 * H + 128 * c:2 * H + 128 * c + 128]
        wqkvT = np.ascontiguousarray(np.concatenate([wq, wk, wv], 0).T).astype(BF16_NP)
        bq = np.stack(
            [qkv_b[fg * H + 128 * c:fg * H + 128 * c + 128] for fg in range(3)],
            axis=1,
        ).astype(np.float32)
        woTa = np.zeros((HPC, HD + 1, H + 1), np.float32)
        for h in range(HPC):
            g = HPC * c + h
            w = out_w[:, g * HD:(g + 1) * HD].T  # [64, 1024]
            if h == 0:
                # head A: colsum is row 0 of o_aug (ones col precedes v)
                woTa[h, 1:HD + 1, 0:H] = w
                woTa[h, 0, H] = 1.0
            else:
                # head B: colsum is row 64 (ones col follows v)
                woTa[h, 0:HD, 0:H] = w
                woTa[h, HD, H] = 1.0
        in_maps.append({
            "xT": xT,
            "wqkvT": wqkvT,
            "bqkv": np.ascontiguousarray(bq),
            "woTa": woTa.astype(BF16_NP),
        })
    return in_maps


def _run(in_maps, trace=False):
    # The image's antenv lacks axon_hooks; register the NTFF profile hook so
    # run_bass_kernel_spmd(trace=True) can report exec_time_ns.
    if trace and "antenv.axon_hooks" not in sys.modules:
        try:
            import trn_agent_boot.trn_boot as _tb
            _hook = _tb._ntff_profile_via_ctypes("/opt/axon/libaxon_pjrt.so")
            _m = types.ModuleType("antenv.axon_hooks")
            _m.get_axon_ntff_profile_hook = lambda: _hook
            sys.modules["antenv.axon_hooks"] = _m
        except Exception:
            trace = False
    from concourse.bass_utils import run_bass_kernel_spmd

    nc = _get_nc()
    res = run_bass_kernel_spmd(nc, in_maps, core_ids=list(range(NCORES)), trace=trace)
    return res


def kernel(x, qkv_w, qkv_b, out_w, out_b):
    in_maps = _host_prep(x, qkv_w, qkv_b, out_w)
    res = _run(in_maps, trace=False)
    total = np.zeros((T, H), np.float32)
    for c in range(NCORES):
        total += res.results[c]["out"]
    total += np.asarray(out_b, dtype=np.float32)[None, :]
    return total.reshape(B, S, H)


# revision 7
# speedup vs baseline: 1.3833x; 1.1620x over previous
"""Fused multi-head attention on 8 Trainium2 NeuronCores.

Problem: x[2,2048,1024] -> qkv proj (16 heads, hd=64) -> softmax attention
-> out proj.  Sharding: tensor parallel over heads, 2 heads per core.
Each core computes q/k/v for its 2 heads, full attention for its
4 (batch, head) pairs, and the partial out-projection contribution of its
128 head-dims.  Host sums the 8 partial outputs and adds out_b.

Layouts on device (per core):
  xT    [1024, 4096]  bf16   hidden on partitions, tokens free (b-major)
  qkvT  [128, 4096]   bf16   per group; head A dims on partitions 0-63, B on 64-127
  scores^T in PSUM: [k-tile 128, q 512] per head, heads packed side by side
  p = exp(scores/8) in SBUF bf16 (no max subtraction: |scores/8| < ~3)
  PV: lhsT = v_aug [k-tile 128, 65] slices of a combined per-batch tile
      holding [pad | onesA | v(A,B) transposed | onesB]; the ones column
      produces the softmax colsum as an extra row of o (row 0 for head A,
      row 64 for head B).
  out-proj per head: lhsT = o_aug^T [65, t-tile 128], rhs = woT_aug [65, 1025]
    (the unit row of rhs is zero in cols 0-1023 and 1 in col 1024, so col
     1024 of the output is the per-token colsum, transposed to partitions)
  y = recipA * y_A + recipB * y_B on DVE (per-partition scalars), f32 out.

The attention loop is software-pipelined: PV lags one k-tile behind
scores/exp, and the previous q-chunk's out-projection epilogue is
interleaved into the current chunk's k-loop so the PE never waits on the
scalar engine's exp.
"""

import sys
import types
import numpy as np
import ml_dtypes

import concourse.bass as bass
import concourse.tile as tile
from concourse import bacc, mybir

BF16 = mybir.dt.bfloat16
F32 = mybir.dt.float32
BF16_NP = ml_dtypes.bfloat16

B, S, H, NH, HD = 2, 2048, 1024, 16, 64
T = B * S               # 4096 tokens, b-major
NCORES = 8
HPC = NH // NCORES      # heads per core = 2
DPC = HPC * HD          # head dims per core = 128
KT = 128                # keys per k-tile
NKT = S // KT           # 16
QC = 512                # query chunk
NQC = S // QC           # 4
HKT = H // 128          # hidden k-tiles = 8
VS = 160                # v_aug stride per k-tile (pad15|onesA|v 128|onesB|pad15)
VOFF = 16               # col offset of the transposed v block within a stride
EXPSCALE = 1.0 / np.sqrt(HD)

_CACHED = {}


def _build_nc():
    nc = bacc.Bacc(None, target_bir_lowering=False, debug=False)
    xT = nc.dram_tensor("xT", [H, T], BF16, kind="ExternalInput").ap()
    wqkvT = nc.dram_tensor("wqkvT", [H, 3 * DPC], BF16, kind="ExternalInput").ap()
    bqkv = nc.dram_tensor("bqkv", [DPC, 3], F32, kind="ExternalInput").ap()
    woTa = nc.dram_tensor("woTa", [HPC, HD + 1, H + 1], BF16, kind="ExternalInput").ap()
    vbias = nc.dram_tensor("vbias", [128, DPC], F32, kind="ExternalInput").ap()
    out = nc.dram_tensor("out", [T, H], F32, kind="ExternalOutput").ap()

    EXP = mybir.ActivationFunctionType.Exp
    MULT = mybir.AluOpType.mult
    ADD = mybir.AluOpType.add

    with tile.TileContext(nc) as tc:
        with (
            tc.tile_pool(name="const", bufs=1) as constp,
            tc.tile_pool(name="xw", bufs=1) as xwp,
            tc.tile_pool(name="qkv", bufs=1) as qkvp,
            tc.tile_pool(name="vaug", bufs=1) as vaugp,
            tc.tile_pool(name="oT", bufs=4) as oTp,
            tc.tile_pool(name="p", bufs=3) as pp,
            tc.tile_pool(name="ysb", bufs=3) as ysbp,
            tc.tile_pool(name="small", bufs=4) as smallp,
            tc.tile_pool(name="ps", bufs=2, space="PSUM") as psp,
        ):
            # ---- constants / weights in ----
            bias_sb = constp.tile([DPC, 3], F32, tag="bias")
            nc.sync.dma_start(bias_sb[:], bqkv[:])
            vbias_sb = constp.tile([128, DPC], F32, tag="vbias")
            nc.sync.dma_start(vbias_sb[:], vbias[:])
            wo_sb = [
                constp.tile([HD + 1, H + 1], BF16, name=f"wo{h}", tag=f"wo{h}")
                for h in range(HPC)
            ]
            for h in range(HPC):
                nc.sync.dma_start(wo_sb[h][:], woTa[h])

            # ---- x and qkv weights in ----
            xT_sb = [xwp.tile([128, T], BF16, name=f"xsb{k}", tag=f"x{k}")
                     for k in range(HKT)]
            wq_sb = [xwp.tile([128, 3 * DPC], BF16, name=f"wsb{k}", tag=f"w{k}")
                     for k in range(HKT)]
            for k in range(HKT):
                nc.sync.dma_start(wq_sb[k][:], wqkvT[k * 128:(k + 1) * 128, :])
                nc.sync.dma_start(xT_sb[k][:], xT[k * 128:(k + 1) * 128, :])

            # vaug tiles (memset to 1.0 early so the ones columns are ready)
            vaug = {}
            for b in range(B):
                va = vaugp.tile([128, NKT * VS], BF16, name=f"va{b}", tag=f"va{b}")
                nc.vector.memset(va[:], 1.0)
                vaug[b] = va

            # ---- qkv projection ----
            # v is computed directly in token-major layout (x^T stationary),
            # written straight into the v_aug tiles; q/k are feature-major,
            # weight-stationary, split by batch so batch-0 attention starts
            # early.
            qkvT_sb = {
                fg: qkvp.tile([128, T], BF16, name=f"qkvsb{fg}", tag=f"qkv{fg}")
                for fg in range(2)
            }
            ADDOP = mybir.AluOpType.add

            def v_group(b):
                for kt in range(NKT):
                    tg = ("o", "y")[kt % 2]
                    v_ps = psp.tile([128, DPC], F32, name=f"vps{b}{kt}", tag=tg)
                    for k in range(HKT):
                        nc.tensor.matmul(
                            v_ps[:],
                            lhsT=xT_sb[k][:, b * S + kt * KT:b * S + (kt + 1) * KT],
                            rhs=wq_sb[k][:, 2 * DPC:3 * DPC],
                            start=(k == 0), stop=(k == HKT - 1),
                        )
                    nc.vector.tensor_add(
                        vaug[b][:, kt * VS + VOFF:kt * VS + VOFF + DPC],
                        v_ps[:], vbias_sb[:],
                    )

            def qk_group(fg, half):
                tiles = [
                    psp.tile([128, 1024], F32, name=f"qp{fg}{half}a", tag="s"),
                    psp.tile([128, 512], F32, name=f"qp{fg}{half}c", tag="o"),
                    psp.tile([128, 512], F32, name=f"qp{fg}{half}d", tag="y"),
                ]

                def tc_slice(t):
                    if t < 2:
                        return tiles[0][:, t * 512:(t + 1) * 512]
                    return tiles[t - 1][:]

                for k in range(HKT):
                    for t in range(4):
                        nc.tensor.matmul(
                            tc_slice(t),
                            lhsT=wq_sb[k][:, fg * DPC:(fg + 1) * DPC],
                            rhs=xT_sb[k][:, half * S + t * 512:half * S + (t + 1) * 512],
                            start=(k == 0),
                            stop=(k == HKT - 1),
                        )
                for t in range(4):
                    nc.vector.tensor_scalar_add(
                        qkvT_sb[fg][:, half * S + t * 512:half * S + (t + 1) * 512],
                        tc_slice(t), bias_sb[:, fg:fg + 1],
                    )

            v_group(0)
            qk_group(0, 0)
            qk_group(1, 0)
            v_group(1)
            qk_group(0, 1)
            qk_group(1, 1)

            qT_sb, kT_sb = qkvT_sb[0], qkvT_sb[1]

            def va_lhsT(b, h, kt):
                # head 0: [onesA | vA] cols 15..79 ; head 1: [vB | onesB] 80..144
                c0 = kt * VS + (15 if h == 0 else 80)
                return vaug[b][:, c0:c0 + HD + 1]

            # ---- attention + pipelined out-projection epilogue ----
            pending = []  # epilogue steps of the previous (b, qc)

            def make_epilogue(b, qc, oT):
                q0 = b * S + qc * QC
                state = {}

                def ct_step():
                    ct = [psp.tile([128, 4], F32, name=f"ct{b}{qc}{h}", tag="y")
                          for h in range(HPC)]
                    rT = smallp.tile([128, 2 * 4], F32, name=f"rT{b}{qc}", tag="rT")
                    for h in range(HPC):
                        for tt in range(4):
                            nc.tensor.matmul(
                                ct[h][:, tt:tt + 1],
                                lhsT=oT[h][:, tt * KT:(tt + 1) * KT],
                                rhs=wo_sb[h][:, H:H + 1],
                                start=True, stop=True,
                            )
                        nc.vector.reciprocal(rT[:, h * 4:h * 4 + 4], ct[h][:])
                    state["rT"] = rT

                steps = [ct_step]

                def y_step(tt, ec):
                    def run():
                        rT = state["rT"]
                        y_ps = [
                            psp.tile([128, 512], F32, name=f"y{b}{qc}{tt}{ec}{h}",
                                     tag="y")
                            for h in range(HPC)
                        ]
                        for h in range(HPC):
                            nc.tensor.matmul(
                                y_ps[h][:],
                                lhsT=oT[h][:, tt * KT:(tt + 1) * KT],
                                rhs=wo_sb[h][:, ec * 512:(ec + 1) * 512],
                                start=True, stop=True,
                            )
                        y_sb = ysbp.tile([128, 512], F32, name=f"ys{b}{qc}{tt}{ec}",
                                         tag="ysb")
                        nc.vector.tensor_scalar_mul(
                            y_sb[:], y_ps[0][:], rT[:, 0 * 4 + tt:0 * 4 + tt + 1])
                        nc.vector.scalar_tensor_tensor(
                            y_sb[:], y_ps[1][:], rT[:, 1 * 4 + tt:1 * 4 + tt + 1],
                            y_sb[:], op0=MULT, op1=ADD,
                        )
                        nc.sync.dma_start(
                            out[q0 + tt * KT:q0 + (tt + 1) * KT,
                                ec * 512:(ec + 1) * 512],
                            y_sb[:],
                        )
                    return run

                for tt in range(4):
                    for ec in range(2):
                        steps.append(y_step(tt, ec))
                return steps

            for b in range(B):
                for qc in range(NQC):
                    q0 = b * S + qc * QC
                    o_ps = [psp.tile([HD + 1, QC], F32, name=f"o{b}{qc}{h}", tag="o")
                            for h in range(HPC)]
                    p_tiles = []
                    for kt in range(NKT):
                        s_ps = psp.tile([128, HPC * QC], F32, tag="s")
                        for h in range(HPC):
                            nc.tensor.matmul(
                                s_ps[:, h * QC:(h + 1) * QC],
                                lhsT=kT_sb[h * HD:(h + 1) * HD,
                                           b * S + kt * KT:b * S + (kt + 1) * KT],
                                rhs=qT_sb[h * HD:(h + 1) * HD, q0:q0 + QC],
                                start=True, stop=True,
                            )
                        p_sb = pp.tile([128, HPC * QC], BF16, tag="p")
                        nc.scalar.activation(p_sb[:], s_ps[:], EXP,
                                             scale=float(EXPSCALE))
                        p_tiles.append(p_sb)
                        if pending:
                            pending.pop(0)()
                        if kt >= 2:
                            for h in range(HPC):
                                nc.tensor.matmul(
                                    o_ps[h][:],
                                    lhsT=va_lhsT(b, h, kt - 2),
                                    rhs=p_tiles[kt - 2][:, h * QC:(h + 1) * QC],
                                    start=(kt - 2 == 0), stop=False,
                                )
                    for lag in (NKT - 2, NKT - 1):
                        for h in range(HPC):
                            nc.tensor.matmul(
                                o_ps[h][:],
                                lhsT=va_lhsT(b, h, lag),
                                rhs=p_tiles[lag][:, h * QC:(h + 1) * QC],
                                start=False, stop=(lag == NKT - 1),
                            )
                    oT = [oTp.tile([HD + 1, QC], BF16, name=f"oT{b}{qc}{h}", tag="oT")
                          for h in range(HPC)]
                    for h in range(HPC):
                        nc.vector.tensor_copy(oT[h][:], o_ps[h][:])
                    while pending:
                        pending.pop(0)()
                    pending = make_epilogue(b, qc, oT)
            while pending:
                pending.pop(0)()
    nc.compile()
    return nc


def _get_nc():
    if "nc" not in _CACHED:
        _CACHED["nc"] = _build_nc()
    return _CACHED["nc"]


def _host_prep(x, qkv_w, qkv_b, out_w):
    x = np.asarray(x, dtype=np.float32)
    qkv_w = np.asarray(qkv_w, dtype=np.float32)
    qkv_b = np.asarray(qkv_b, dtype=np.float32)
    out_w = np.asarray(out_w, dtype=np.float32)

    xT = np.ascontiguousarray(x.reshape(T, H).T).astype(BF16_NP)
    in_maps = []
    for c in range(NCORES):
        wq = qkv_w[128 * c:128 * c + 128]
        wk = qkv_w[H + 128 * c:H + 128 * c + 128]
        wv = qkv_w[2 * H + 128 * c:2 * H + 128 * c + 128]
        wqkvT = np.ascontiguousarray(np.concatenate([wq, wk, wv], 0).T).astype(BF16_NP)
        bq = np.stack(
            [qkv_b[fg * H + 128 * c:fg * H + 128 * c + 128] for fg in range(3)],
            axis=1,
        ).astype(np.float32)
        woTa = np.zeros((HPC, HD + 1, H + 1), np.float32)
        for h in range(HPC):
            g = HPC * c + h
            w = out_w[:, g * HD:(g + 1) * HD].T  # [64, 1024]
            if h == 0:
                # head A: colsum is row 0 of o_aug (ones col precedes v)
                woTa[h, 1:HD + 1, 0:H] = w
                woTa[h, 0, H] = 1.0
            else:
                # head B: colsum is row 64 (ones col follows v)
                woTa[h, 0:HD, 0:H] = w
                woTa[h, HD, H] = 1.0
        vb = np.broadcast_to(
            qkv_b[2 * H + 128 * c:2 * H + 128 * c + 128][None, :], (128, DPC)
        ).astype(np.float32)
        in_maps.append({
            "xT": xT,
            "wqkvT": wqkvT,
            "bqkv": np.ascontiguousarray(bq),
            "woTa": woTa.astype(BF16_NP),
            "vbias": np.ascontiguousarray(vb),
        })
    return in_maps


def _run(in_maps, trace=False):
    # The image's antenv lacks axon_hooks; register the NTFF profile hook so
    # run_bass_kernel_spmd(trace=True) can report exec_time_ns.
    if trace and "antenv.axon_hooks" not in sys.modules:
        try:
            import trn_agent_boot.trn_boot as _tb
            _hook = _tb._ntff_profile_via_ctypes("/opt/axon/libaxon_pjrt.so")
            _m = types.ModuleType("antenv.axon_hooks")
            _m.get_axon_ntff_profile_hook = lambda: _hook
            sys.modules["antenv.axon_hooks"] = _m
        except Exception:
            trace = False
    from concourse.bass_utils import run_bass_kernel_spmd

    nc = _get_nc()
    res = run_bass_kernel_spmd(nc, in_maps, core_ids=list(range(NCORES)), trace=trace)
    return res


def kernel(x, qkv_w, qkv_b, out_w, out_b):
    in_maps = _host_prep(x, qkv_w, qkv_b, out_w)
    res = _run(in_maps, trace=False)
    total = np.zeros((T, H), np.float32)
    for c in range(NCORES):
        total += res.results[c]["out"]
    total += np.asarray(out_b, dtype=np.float32)[None, :]
    return total.reshape(B, S, H)
